# revision 19
# baseline (speedup 1.0000x reference)
"""ChainCRF negative log-likelihood on 8 Trainium2 NeuronCores.

Reference computation (per batch element b):
    part_0 = e[0][64, :]                      (e = energy * mask)
    part_t = logsumexp_i(e[t][i, j] + part_{t-1}[i])   (gated by mask)
    tgt    = sum_t e[t][prev_t, cur_t]
    loss_b = logsumexp_j(part_L[j]) - tgt

Device algorithm (linear domain, constant per-step rescale; the log-drift
of the running sums is a bounded random walk, measured within [-1.5, +0.7]
over 512 steps for N(0,1) energies, so no per-step normalization is needed):

    E_t = exp(ce_t - c),  c = log(65) + 0.5
    forward  half:  u_t = E_t^T u_{t-1},  u_{-1} = onehot(64),  t = 0..255
    backward half:  w_{t-1} = E_t w_t,    w_{511} = ones,       t = 511..256
    S_b    = u_255^T w_255                (splits the serial chain in two)
    loss_b = ln(S_b) + 512*c - tgt

Sharding: pure data parallel, 4 batch elements per core, no collectives.

Layout: host transposes energies to [i, t, b, j] fp16 for the forward half
and [j, t, b, i] for the backward half, so every (t, b) slice is a
ready-to-use matmul lhsT [K(65 partitions), M(65)] and each chunk of
timesteps is one contiguous DMA.  Per step and direction: 4 matmuls
(stationary = exp'd energy, moving = state column [65, 1], one PSUM bank
per step) then ONE DVE copy PSUM->SBUF [65, 4].  The 8 chains (4 batch x
2 directions) are independent, so their serial MM->copy->MM latencies
overlap across engines.

tgt path: host precomputes flat gather indices, device gathers via
indirect DMA, masks, reduces, and folds into the loss.
"""

import os
import numpy as np
from contextlib import ExitStack

B, L, NL = 32, 512, 65
H = L // 2
NCORES = 8
BPC = B // NCORES                      # batch per core = 4
CH = 16                                # timesteps per DMA/exp chunk
CBIAS = float(np.float32(np.log(NL) + 0.5))

# gather arrangement: 2048 = 128 partitions x 16 cols;
# partition p = b*32 + r, col g, with t = g*32 + r
GCOLS = L // 32                        # 16

_CACHE = {}

# populated by the last kernel() call when CRF_TRACE=1
last_exec_ns = None
last_profile = None


def _build_program():
    from concourse import bacc, mybir, tile
    import concourse.bass as bass

    f16 = mybir.dt.float16
    f32 = mybir.dt.float32
    i32 = mybir.dt.int32
    Alu = mybir.AluOpType
    Act = mybir.ActivationFunctionType

    nc = bacc.Bacc("TRN2", target_bir_lowering=False, debug=False,
                   num_devices=NCORES)

    energy_h = nc.dram_tensor("energy", [NL, L, BPC, NL], f16,
                              kind="ExternalInput")
    gidx_h = nc.dram_tensor("gidx", [128, GCOLS], i32, kind="ExternalInput")
    gmask_h = nc.dram_tensor("gmask", [128, GCOLS], f32, kind="ExternalInput")
    initu_h = nc.dram_tensor("initu", [NL, BPC], f16, kind="ExternalInput")
    initw_h = nc.dram_tensor("initw", [NL, BPC], f16, kind="ExternalInput")
    bones_h = nc.dram_tensor("bones", [128, BPC], f32, kind="ExternalInput")
    loss_h = nc.dram_tensor("loss", [1, BPC], f32, kind="ExternalOutput")

    energy = energy_h.ap()

    with tile.TileContext(nc) as tc, ExitStack() as ctx:
        cpool = ctx.enter_context(tc.tile_pool(name="consts", bufs=1))
        erawf = ctx.enter_context(tc.tile_pool(name="erawf", bufs=5))
        eexpf = ctx.enter_context(tc.tile_pool(name="eexpf", bufs=4))
        erawb = ctx.enter_context(tc.tile_pool(name="erawb", bufs=5))
        eexpb = ctx.enter_context(tc.tile_pool(name="eexpb", bufs=4))
        upool = ctx.enter_context(tc.tile_pool(name="u", bufs=2))
        wpool = ctx.enter_context(tc.tile_pool(name="w", bufs=2))
        psf = ctx.enter_context(tc.tile_pool(name="psf", bufs=3, space="PSUM"))
        psb = ctx.enter_context(tc.tile_pool(name="psb", bufs=3, space="PSUM"))
        psaux = ctx.enter_context(tc.tile_pool(name="psaux", bufs=1, space="PSUM"))

        initu_t = cpool.tile([NL, BPC], f16)
        nc.sync.dma_start(out=initu_t[:], in_=initu_h.ap())
        initw_t = cpool.tile([NL, BPC], f16)
        nc.sync.dma_start(out=initw_t[:], in_=initw_h.ap())
        bones_t = cpool.tile([128, BPC], f32)
        nc.sync.dma_start(out=bones_t[:], in_=bones_h.ap())
        bias_t = cpool.tile([NL, 1], f32)
        nc.vector.memset(bias_t[:], -CBIAS)

        # ---- main recurrence: 2 independent chain groups (fwd u, bwd w) ----
        # Chunk sizes: small priming chunks so the first matmuls start as
        # early as possible, then steady CH-sized chunks.
        sizes = [2, 2, 4, 8] + [CH] * ((H - 16) // CH)
        assert sum(sizes) == H
        starts = list(np.cumsum([0] + sizes[:-1]))

        def pieces_fwd(size):
            # exp piece boundaries (local t, ascending consumption)
            return [(0, size // 2), (size // 2, size - size // 2)] \
                if size >= 8 else [(0, size)]

        def pieces_bwd(size):
            # descending consumption; asymmetric split so bwd exp
            # boundaries do not line up with fwd ones (de-synchronizes
            # the two chains' stall points on the shared DVE queue)
            if size < 8:
                return [(0, size)]
            q = size // 4
            return [(size - q, q), (q, size - 2 * q), (0, q)]

        u_t = lambda b: initu_t[:, b:b + 1]
        w_t = lambda b: initw_t[:, b:b + 1]
        for c, (t0, size) in enumerate(zip(starts, sizes)):
            erf = erawf.tile([NL, size, BPC, NL], f16, tag="erf")
            nc.sync.dma_start(out=erf[:], in_=energy[:, t0:t0 + size, :, :])
            fw_pieces = []                       # (lo, hi, tile)
            for pi, (lo, ln) in enumerate(pieces_fwd(size)):
                ee = eexpf.tile([NL, ln, BPC, NL], f16, tag=f"eef{pi}")
                nc.scalar.activation(ee[:], erf[:, lo:lo + ln], Act.Exp,
                                     bias=bias_t[:], scale=1.0)
                fw_pieces.append((lo, lo + ln, ee))

            bt0 = L - t0 - size
            erb = erawb.tile([NL, size, BPC, NL], f16, tag="erb")
            nc.sync.dma_start(out=erb[:], in_=energy[:, bt0:bt0 + size, :, :])
            bw_pieces = []
            for pi, (lo, ln) in enumerate(pieces_bwd(size)):
                ee = eexpb.tile([NL, ln, BPC, NL], f16, tag=f"eeb{pi}")
                nc.scalar.activation(ee[:], erb[:, lo:lo + ln], Act.Exp,
                                     bias=bias_t[:], scale=1.0)
                bw_pieces.append((lo, lo + ln, ee))

            def piece_at(pieces, idx):
                for lo, hi, tl in pieces:
                    if lo <= idx < hi:
                        return tl, idx - lo
                raise AssertionError

            for s in range(size):
                # Two groups, each pairing 2 fwd chains with 2 bwd chains in
                # one PSUM bank + one cast.  The fixed A,B cast rotation on
                # the DVE FIFO pins a deterministic phase (no drift into the
                # serialized attractor).
                eef, sf = piece_at(fw_pieces, s)
                eeb, sbl = piece_at(bw_pieces, size - 1 - s)
                psA = psf.tile([NL, 4], f32)
                psB = psb.tile([NL, 4], f32)
                for g, ps in ((0, psA), (1, psB)):
                    for i in range(2):
                        b = 2 * g + i
                        nc.tensor.matmul(ps[:, i:i + 1], lhsT=eef[:, sf, b, :],
                                         rhs=u_t(b), start=True, stop=True)
                    for i in range(2):
                        b = 2 * g + i
                        nc.tensor.matmul(ps[:, 2 + i:3 + i],
                                         lhsT=eeb[:, sbl, b, :],
                                         rhs=w_t(b), start=True, stop=True)
                uwA = upool.tile([NL, 4], f16)
                nc.vector.tensor_copy(out=uwA[:], in_=psA[:])
                uwB = wpool.tile([NL, 4], f16)
                nc.vector.tensor_copy(out=uwB[:], in_=psB[:])

                def mk_state(uw_a, uw_b):
                    def u_fn(b):
                        t = uw_a if b < 2 else uw_b
                        return t[:, (b % 2):(b % 2) + 1]
                    def w_fn(b):
                        t = uw_a if b < 2 else uw_b
                        return t[:, 2 + (b % 2):3 + (b % 2)]
                    return u_fn, w_fn
                u_t, w_t = mk_state(uwA, uwB)

            if c == 8:
                # ---- target-energy path: gather 4x512 scalars, mask, reduce
                # The gidx DMA is enqueued here (sync-queue DMAs run in queue
                # order) so the indirect gathers cannot start during the
                # pipeline ramp-up and steal SDMA throughput.
                gidx_t = cpool.tile([128, GCOLS], i32)
                nc.sync.dma_start(out=gidx_t[:], in_=gidx_h.ap())
                gmask_t = cpool.tile([128, GCOLS], f32)
                nc.sync.dma_start(out=gmask_t[:], in_=gmask_h.ap())
                eflat = energy.rearrange("i t b j -> (i t b j)")[:, None]
                gath_t = cpool.tile([128, GCOLS], f16)
                for g in range(GCOLS):
                    nc.gpsimd.indirect_dma_start(
                        out=gath_t[:, g:g + 1],
                        out_offset=None,
                        in_=eflat,
                        in_offset=bass.IndirectOffsetOnAxis(
                            ap=gidx_t[:, g:g + 1], axis=0),
                    )
                gm_t = cpool.tile([128, GCOLS], f32)
                nc.vector.tensor_tensor(out=gm_t[:], in0=gath_t[:],
                                        in1=gmask_t[:], op=Alu.mult)
                gred_t = cpool.tile([128, 1], f32)
                nc.vector.tensor_reduce(out=gred_t[:], in_=gm_t[:],
                                        axis=mybir.AxisListType.X, op=Alu.add)
                tgt_ps = psaux.tile([1, BPC], f32)
                nc.tensor.matmul(tgt_ps[:], lhsT=gred_t[:], rhs=bones_t[:],
                                 start=True, stop=True)
                tgt_sb = cpool.tile([1, BPC], f32)
                nc.vector.tensor_copy(out=tgt_sb[:], in_=tgt_ps[:])

        # ---- epilogue: loss = ln(u^T w) + L*c - tgt ----
        s_ps = psaux.tile([1, BPC], f32)
        for b in range(BPC):
            nc.tensor.matmul(s_ps[:, b:b + 1], lhsT=u_t(b),
                             rhs=w_t(b), start=True, stop=True)
        lsb = cpool.tile([1, BPC], f32)
        nc.scalar.activation(lsb[:], s_ps[:], Act.Ln, bias=0.0, scale=1.0)
        nc.vector.tensor_tensor(out=lsb[:], in0=lsb[:], in1=tgt_sb[:],
                                op=Alu.subtract)
        nc.vector.tensor_scalar_add(lsb[:], lsb[:], float(L) * CBIAS)
        nc.sync.dma_start(out=loss_h.ap(), in_=lsb[:])

    nc.compile()
    return nc


def _get_program():
    if "nc" not in _CACHE:
        _CACHE["nc"] = _build_program()
    return _CACHE["nc"]


def _prep_inputs(energy, target, mask):
    """Host-side sharding + layout. Returns in_maps (one dict per core)."""
    energy = np.asarray(energy, dtype=np.float32)
    target = np.asarray(target).astype(np.int64)
    mask = np.asarray(mask, dtype=np.float32)

    all_ones = bool(np.all(mask == 1.0))
    if all_ones:
        energy_eff = energy
        gmask_full = np.ones((B, L), np.float32)
    else:
        # binary-mask general path: masked steps (t>0) become identity
        # transitions after exp(x - c); masked t=0 stays the zero matrix.
        energy_eff = energy * mask[:, :, None, None]
        sub = np.full((NL, NL), -1e4, np.float32)
        np.fill_diagonal(sub, CBIAS)
        zb, zt = np.nonzero(mask == 0.0)
        for bb, tt in zip(zb, zt):
            if tt > 0:
                energy_eff[bb, tt] = sub
        gmask_full = (mask != 0.0).astype(np.float32)

    initu = np.zeros((NL, BPC), np.float16)
    initu[NL - 1, :] = 1.0
    initw = np.ones((NL, BPC), np.float16)
    bones = np.zeros((128, BPC), np.float32)
    for b in range(BPC):
        bones[b * 32:(b + 1) * 32, b] = 1.0

    in_maps = []
    for k in range(NCORES):
        sl = slice(k * BPC, (k + 1) * BPC)
        eb = energy_eff[sl]                                   # [4, L, 65, 65]
        et = np.empty((NL, L, BPC, NL), np.float16)
        et[:, :H] = eb[:, :H].transpose(2, 1, 0, 3)           # fwd: [i, t, b, j]
        et[:, H:] = eb[:, H:].transpose(3, 1, 0, 2)           # bwd: [j, t, b, i]

        tg = target[sl]                                       # [4, L]
        mk = gmask_full[sl]
        prev = np.concatenate(
            [np.full((BPC, 1), NL - 1, np.int64), tg[:, :-1]], axis=1)
        tt = np.arange(L, dtype=np.int64)[None, :]
        bb = np.arange(BPC, dtype=np.int64)[:, None]
        # fwd layout [i=prev, t, b, j=cur]; bwd layout [j=cur, t, b, i=prev]
        flat_f = ((prev * L + tt) * BPC + bb) * NL + tg
        flat_b = ((tg * L + tt) * BPC + bb) * NL + prev
        flat = np.where(tt < H, flat_f, flat_b)               # [4, L]

        gidx = np.zeros((128, GCOLS), np.int32)
        gmask = np.zeros((128, GCOLS), np.float32)
        for b in range(BPC):
            # partition p = b*32 + r holds t = g*32 + r at column g
            gidx[b * 32:(b + 1) * 32, :] = flat[b].reshape(GCOLS, 32).T
            gmask[b * 32:(b + 1) * 32, :] = mk[b].reshape(GCOLS, 32).T

        in_maps.append({
            "energy": et,
            "gidx": gidx,
            "gmask": gmask,
            "initu": initu,
            "initw": initw,
            "bones": bones,
        })
    return in_maps


def _install_ntff_hook_shim():
    """The agent image's antenv lacks axon_hooks; synthesize it so
    run_bass_kernel_spmd(trace=True) can find the NTFF profile hook."""
    import sys
    import types
    try:
        import antenv.axon_hooks  # noqa: F401
        return
    except ImportError:
        pass
    import antenv
    mod = types.ModuleType("antenv.axon_hooks")
    _h = [None]
    mod.set_axon_ntff_profile_hook = lambda h: _h.__setitem__(0, h)
    mod.get_axon_ntff_profile_hook = lambda: _h[0]
    sys.modules["antenv.axon_hooks"] = mod
    antenv.axon_hooks = mod
    try:
        from trn_agent_boot.trn_boot import _ntff_profile_via_ctypes
        hook = _ntff_profile_via_ctypes("/opt/axon/libaxon_pjrt.so")
        if hook is not None:
            mod.set_axon_ntff_profile_hook(hook)
    except Exception:
        pass


def kernel(energy, target, mask):
    global last_exec_ns, last_profile
    from concourse.bass_utils import run_bass_kernel_spmd

    nc = _get_program()
    in_maps = _prep_inputs(energy, target, mask)
    trace = bool(int(os.environ.get("CRF_TRACE", "0")))
    if trace:
        _install_ntff_hook_shim()
    res = run_bass_kernel_spmd(nc, in_maps, list(range(NCORES)), trace=trace)
    last_exec_ns = res.exec_time_ns
    last_profile = res.profile_json
    out = np.concatenate(
        [res.results[k]["loss"].reshape(BPC) for k in range(NCORES)])
    return out.astype(np.float32)


# revision 21
# speedup vs baseline: 1.0251x; 1.0251x over previous
"""ChainCRF negative log-likelihood on 8 Trainium2 NeuronCores.

Reference computation (per batch element b):
    part_0 = e[0][64, :]                      (e = energy * mask)
    part_t = logsumexp_i(e[t][i, j] + part_{t-1}[i])   (gated by mask)
    tgt    = sum_t e[t][prev_t, cur_t]
    loss_b = logsumexp_j(part_L[j]) - tgt

Device algorithm (linear domain, constant per-step rescale; the log-drift
of the running sums is a bounded random walk, measured within [-1.5, +0.7]
over 512 steps for N(0,1) energies, so no per-step normalization is needed):

    E_t = exp(ce_t - c),  c = log(65) + 0.5
    forward  half:  u_t = E_t^T u_{t-1},  u_{-1} = onehot(64),  t = 0..255
    backward half:  w_{t-1} = E_t w_t,    w_{511} = ones,       t = 511..256
    S_b    = u_255^T w_255                (splits the serial chain in two)
    loss_b = ln(S_b) + 512*c - tgt

Sharding: pure data parallel, 4 batch elements per core, no collectives.

Layout: host transposes energies to [i, t, b, j] fp16 for the forward half
and [j, t, b, i] for the backward half, so every (t, b) slice is a
ready-to-use matmul lhsT [K(65 partitions), M(65)] and each chunk of
timesteps is one contiguous DMA.  Per step and direction: 4 matmuls
(stationary = exp'd energy, moving = state column [65, 1], one PSUM bank
per step) then ONE DVE copy PSUM->SBUF [65, 4].  The 8 chains (4 batch x
2 directions) are independent, so their serial MM->copy->MM latencies
overlap across engines.

tgt path: host precomputes flat gather indices, device gathers via
indirect DMA, masks, reduces, and folds into the loss.
"""

import os
import numpy as np
from contextlib import ExitStack

B, L, NL = 32, 512, 65
H = L // 2
NCORES = 8
BPC = B // NCORES                      # batch per core = 4
CH = 16                                # timesteps per DMA/exp chunk
CBIAS = float(np.float32(np.log(NL) + 0.5))

# gather arrangement: 2048 = 128 partitions x 16 cols;
# partition p = b*32 + r, col g, with t = g*32 + r
GCOLS = L // 32                        # 16

_CACHE = {}

# populated by the last kernel() call when CRF_TRACE=1
last_exec_ns = None
last_profile = None


def _build_program():
    from concourse import bacc, mybir, tile
    import concourse.bass as bass

    f16 = mybir.dt.float16
    f32 = mybir.dt.float32
    i32 = mybir.dt.int32
    Alu = mybir.AluOpType
    Act = mybir.ActivationFunctionType

    nc = bacc.Bacc("TRN2", target_bir_lowering=False, debug=False,
                   num_devices=NCORES)

    energy_h = nc.dram_tensor("energy", [NL, L, BPC, NL], f16,
                              kind="ExternalInput")
    gidx_h = nc.dram_tensor("gidx", [128, GCOLS], i32, kind="ExternalInput")
    gmask_h = nc.dram_tensor("gmask", [128, GCOLS], f32, kind="ExternalInput")
    initu_h = nc.dram_tensor("initu", [NL, BPC], f16, kind="ExternalInput")
    initw_h = nc.dram_tensor("initw", [NL, BPC], f16, kind="ExternalInput")
    bones_h = nc.dram_tensor("bones", [128, BPC], f32, kind="ExternalInput")
    loss_h = nc.dram_tensor("loss", [1, BPC], f32, kind="ExternalOutput")

    energy = energy_h.ap()

    with tile.TileContext(nc) as tc, ExitStack() as ctx:
        cpool = ctx.enter_context(tc.tile_pool(name="consts", bufs=1))
        erawf = ctx.enter_context(tc.tile_pool(name="erawf", bufs=5))
        eexpf = ctx.enter_context(tc.tile_pool(name="eexpf", bufs=3))
        erawb = ctx.enter_context(tc.tile_pool(name="erawb", bufs=5))
        eexpb = ctx.enter_context(tc.tile_pool(name="eexpb", bufs=3))
        upool = ctx.enter_context(tc.tile_pool(name="u", bufs=2))
        wpool = ctx.enter_context(tc.tile_pool(name="w", bufs=2))
        psf = ctx.enter_context(tc.tile_pool(name="psf", bufs=3, space="PSUM"))
        psb = ctx.enter_context(tc.tile_pool(name="psb", bufs=3, space="PSUM"))
        psaux = ctx.enter_context(tc.tile_pool(name="psaux", bufs=1, space="PSUM"))

        initu_t = cpool.tile([NL, BPC], f16)
        nc.sync.dma_start(out=initu_t[:], in_=initu_h.ap())
        initw_t = cpool.tile([NL, BPC], f16)
        nc.sync.dma_start(out=initw_t[:], in_=initw_h.ap())
        bones_t = cpool.tile([128, BPC], f32)
        nc.sync.dma_start(out=bones_t[:], in_=bones_h.ap())
        bias_t = cpool.tile([NL, 1], f32)
        nc.vector.memset(bias_t[:], -CBIAS)

        # ---- main recurrence: 2 independent chain groups (fwd u, bwd w) ----
        # Chunk sizes: small priming chunks so the first matmuls start as
        # early as possible, then steady CH-sized chunks.
        sizes = [4, 4, 8] + [CH] * ((H - 16) // CH)
        assert sum(sizes) == H
        starts = list(np.cumsum([0] + sizes[:-1]))

        def pieces_fwd(size):
            # exp piece boundaries (local t, ascending consumption)
            return [(0, size // 2), (size // 2, size - size // 2)] \
                if size >= 8 else [(0, size)]

        def pieces_bwd(size):
            # descending consumption; asymmetric split so bwd exp
            # boundaries do not line up with fwd ones (de-synchronizes
            # the two chains' stall points on the shared DVE queue)
            if size < 8:
                return [(0, size)]
            q = size // 4
            return [(size - q, q), (q, size - 2 * q), (0, q)]

        u_t = lambda b: initu_t[:, b:b + 1]
        w_t = lambda b: initw_t[:, b:b + 1]
        for c, (t0, size) in enumerate(zip(starts, sizes)):
            erf = erawf.tile([NL, size, BPC, NL], f16, tag="erf")
            nc.sync.dma_start(out=erf[:], in_=energy[:, t0:t0 + size, :, :])
            fw_pieces = []                       # (lo, hi, tile)
            for pi, (lo, ln) in enumerate(pieces_fwd(size)):
                ee = eexpf.tile([NL, ln, BPC, NL], f16, tag=f"eef{pi}")
                nc.scalar.activation(ee[:], erf[:, lo:lo + ln], Act.Exp,
                                     bias=bias_t[:], scale=1.0)
                fw_pieces.append((lo, lo + ln, ee))

            bt0 = L - t0 - size
            erb = erawb.tile([NL, size, BPC, NL], f16, tag="erb")
            nc.sync.dma_start(out=erb[:], in_=energy[:, bt0:bt0 + size, :, :])
            bw_pieces = []
            for pi, (lo, ln) in enumerate(pieces_bwd(size)):
                ee = eexpb.tile([NL, ln, BPC, NL], f16, tag=f"eeb{pi}")
                nc.scalar.activation(ee[:], erb[:, lo:lo + ln], Act.Exp,
                                     bias=bias_t[:], scale=1.0)
                bw_pieces.append((lo, lo + ln, ee))

            def piece_at(pieces, idx):
                for lo, hi, tl in pieces:
                    if lo <= idx < hi:
                        return tl, idx - lo
                raise AssertionError

            for s in range(size):
                # Two groups, each pairing 2 fwd chains with 2 bwd chains in
                # one PSUM bank + one cast.  The fixed A,B cast rotation on
                # the DVE FIFO pins a deterministic phase (no drift into the
                # serialized attractor).
                eef, sf = piece_at(fw_pieces, s)
                eeb, sbl = piece_at(bw_pieces, size - 1 - s)
                psA = psf.tile([NL, 4], f32)
                psB = psb.tile([NL, 4], f32)
                for g, ps in ((0, psA), (1, psB)):
                    for i in range(2):
                        b = 2 * g + i
                        nc.tensor.matmul(ps[:, i:i + 1], lhsT=eef[:, sf, b, :],
                                         rhs=u_t(b), start=True, stop=True)
                    for i in range(2):
                        b = 2 * g + i
                        nc.tensor.matmul(ps[:, 2 + i:3 + i],
                                         lhsT=eeb[:, sbl, b, :],
                                         rhs=w_t(b), start=True, stop=True)
                uwA = upool.tile([NL, 4], f16)
                nc.vector.tensor_copy(out=uwA[:], in_=psA[:])
                uwB = wpool.tile([NL, 4], f16)
                nc.vector.tensor_copy(out=uwB[:], in_=psB[:])

                def mk_state(uw_a, uw_b):
                    def u_fn(b):
                        t = uw_a if b < 2 else uw_b
                        return t[:, (b % 2):(b % 2) + 1]
                    def w_fn(b):
                        t = uw_a if b < 2 else uw_b
                        return t[:, 2 + (b % 2):3 + (b % 2)]
                    return u_fn, w_fn
                u_t, w_t = mk_state(uwA, uwB)

            if c == 8:
                # ---- target-energy path: gather 4x512 scalars, mask, reduce
                # The gidx DMA is enqueued here (sync-queue DMAs run in queue
                # order) so the indirect gathers cannot start during the
                # pipeline ramp-up and steal SDMA throughput.
                gidx_t = cpool.tile([128, GCOLS], i32)
                nc.sync.dma_start(out=gidx_t[:], in_=gidx_h.ap())
                gmask_t = cpool.tile([128, GCOLS], f32)
                nc.sync.dma_start(out=gmask_t[:], in_=gmask_h.ap())
                eflat = energy.rearrange("i t b j -> (i t b j)")[:, None]
                gath_t = cpool.tile([128, GCOLS], f16)
                for g in range(GCOLS):
                    nc.gpsimd.indirect_dma_start(
                        out=gath_t[:, g:g + 1],
                        out_offset=None,
                        in_=eflat,
                        in_offset=bass.IndirectOffsetOnAxis(
                            ap=gidx_t[:, g:g + 1], axis=0),
                    )
                gm_t = cpool.tile([128, GCOLS], f32)
                nc.vector.tensor_tensor(out=gm_t[:], in0=gath_t[:],
                                        in1=gmask_t[:], op=Alu.mult)
                gred_t = cpool.tile([128, 1], f32)
                nc.vector.tensor_reduce(out=gred_t[:], in_=gm_t[:],
                                        axis=mybir.AxisListType.X, op=Alu.add)
                tgt_ps = psaux.tile([1, BPC], f32)
                nc.tensor.matmul(tgt_ps[:], lhsT=gred_t[:], rhs=bones_t[:],
                                 start=True, stop=True)
                tgt_sb = cpool.tile([1, BPC], f32)
                nc.vector.tensor_copy(out=tgt_sb[:], in_=tgt_ps[:])

        # ---- epilogue: loss = ln(u^T w) + L*c - tgt ----
        s_ps = psaux.tile([1, BPC], f32)
        for b in range(BPC):
            nc.tensor.matmul(s_ps[:, b:b + 1], lhsT=u_t(b),
                             rhs=w_t(b), start=True, stop=True)
        lsb = cpool.tile([1, BPC], f32)
        nc.scalar.activation(lsb[:], s_ps[:], Act.Ln, bias=0.0, scale=1.0)
        nc.vector.tensor_tensor(out=lsb[:], in0=lsb[:], in1=tgt_sb[:],
                                op=Alu.subtract)
        nc.vector.tensor_scalar_add(lsb[:], lsb[:], float(L) * CBIAS)
        nc.sync.dma_start(out=loss_h.ap(), in_=lsb[:])

    nc.compile()
    return nc


def _get_program():
    if "nc" not in _CACHE:
        _CACHE["nc"] = _build_program()
    return _CACHE["nc"]


def _prep_inputs(energy, target, mask):
    """Host-side sharding + layout. Returns in_maps (one dict per core)."""
    energy = np.asarray(energy, dtype=np.float32)
    target = np.asarray(target).astype(np.int64)
    mask = np.asarray(mask, dtype=np.float32)

    all_ones = bool(np.all(mask == 1.0))
    if all_ones:
        energy_eff = energy
        gmask_full = np.ones((B, L), np.float32)
    else:
        # binary-mask general path: masked steps (t>0) become identity
        # transitions after exp(x - c); masked t=0 stays the zero matrix.
        energy_eff = energy * mask[:, :, None, None]
        sub = np.full((NL, NL), -1e4, np.float32)
        np.fill_diagonal(sub, CBIAS)
        zb, zt = np.nonzero(mask == 0.0)
        for bb, tt in zip(zb, zt):
            if tt > 0:
                energy_eff[bb, tt] = sub
        gmask_full = (mask != 0.0).astype(np.float32)

    initu = np.zeros((NL, BPC), np.float16)
    initu[NL - 1, :] = 1.0
    initw = np.ones((NL, BPC), np.float16)
    bones = np.zeros((128, BPC), np.float32)
    for b in range(BPC):
        bones[b * 32:(b + 1) * 32, b] = 1.0

    in_maps = []
    for k in range(NCORES):
        sl = slice(k * BPC, (k + 1) * BPC)
        eb = energy_eff[sl]                                   # [4, L, 65, 65]
        et = np.empty((NL, L, BPC, NL), np.float16)
        et[:, :H] = eb[:, :H].transpose(2, 1, 0, 3)           # fwd: [i, t, b, j]
        et[:, H:] = eb[:, H:].transpose(3, 1, 0, 2)           # bwd: [j, t, b, i]

        tg = target[sl]                                       # [4, L]
        mk = gmask_full[sl]
        prev = np.concatenate(
            [np.full((BPC, 1), NL - 1, np.int64), tg[:, :-1]], axis=1)
        tt = np.arange(L, dtype=np.int64)[None, :]
        bb = np.arange(BPC, dtype=np.int64)[:, None]
        # fwd layout [i=prev, t, b, j=cur]; bwd layout [j=cur, t, b, i=prev]
        flat_f = ((prev * L + tt) * BPC + bb) * NL + tg
        flat_b = ((tg * L + tt) * BPC + bb) * NL + prev
        flat = np.where(tt < H, flat_f, flat_b)               # [4, L]

        gidx = np.zeros((128, GCOLS), np.int32)
        gmask = np.zeros((128, GCOLS), np.float32)
        for b in range(BPC):
            # partition p = b*32 + r holds t = g*32 + r at column g
            gidx[b * 32:(b + 1) * 32, :] = flat[b].reshape(GCOLS, 32).T
            gmask[b * 32:(b + 1) * 32, :] = mk[b].reshape(GCOLS, 32).T

        in_maps.append({
            "energy": et,
            "gidx": gidx,
            "gmask": gmask,
            "initu": initu,
            "initw": initw,
            "bones": bones,
        })
    return in_maps


def _install_ntff_hook_shim():
    """The agent image's antenv lacks axon_hooks; synthesize it so
    run_bass_kernel_spmd(trace=True) can find the NTFF profile hook."""
    import sys
    import types
    try:
        import antenv.axon_hooks  # noqa: F401
        return
    except ImportError:
        pass
    import antenv
    mod = types.ModuleType("antenv.axon_hooks")
    _h = [None]
    mod.set_axon_ntff_profile_hook = lambda h: _h.__setitem__(0, h)
    mod.get_axon_ntff_profile_hook = lambda: _h[0]
    sys.modules["antenv.axon_hooks"] = mod
    antenv.axon_hooks = mod
    try:
        from trn_agent_boot.trn_boot import _ntff_profile_via_ctypes
        hook = _ntff_profile_via_ctypes("/opt/axon/libaxon_pjrt.so")
        if hook is not None:
            mod.set_axon_ntff_profile_hook(hook)
    except Exception:
        pass


def kernel(energy, target, mask):
    global last_exec_ns, last_profile
    from concourse.bass_utils import run_bass_kernel_spmd

    nc = _get_program()
    in_maps = _prep_inputs(energy, target, mask)
    trace = bool(int(os.environ.get("CRF_TRACE", "0")))
    if trace:
        _install_ntff_hook_shim()
    res = run_bass_kernel_spmd(nc, in_maps, list(range(NCORES)), trace=trace)
    last_exec_ns = res.exec_time_ns
    last_profile = res.profile_json
    out = np.concatenate(
        [res.results[k]["loss"].reshape(BPC) for k in range(NCORES)])
    return out.astype(np.float32)


# revision 25
# speedup vs baseline: 1.0793x; 1.0529x over previous
"""ChainCRF negative log-likelihood on 8 Trainium2 NeuronCores.

Reference computation (per batch element b):
    part_0 = e[0][64, :]                      (e = energy * mask)
    part_t = logsumexp_i(e[t][i, j] + part_{t-1}[i])   (gated by mask)
    tgt    = sum_t e[t][prev_t, cur_t]
    loss_b = logsumexp_j(part_L[j]) - tgt

Device algorithm (linear domain, constant per-step rescale; the log-drift
of the running sums is a bounded random walk, measured within [-1.5, +0.7]
over 512 steps for N(0,1) energies, so no per-step normalization is needed):

    E_t = exp(ce_t - c),  c = log(65) + 0.5
    forward  half:  u_t = E_t^T u_{t-1},  u_{-1} = onehot(64),  t = 0..255
    backward half:  w_{t-1} = E_t w_t,    w_{511} = ones,       t = 511..256
    S_b    = u_255^T w_255                (splits the serial chain in two)
    loss_b = ln(S_b) + 512*c - tgt

Sharding: pure data parallel, 4 batch elements per core, no collectives.

Layout: host transposes energies to [i, t, b, j] fp16 for the forward half
and [j, t, b, i] for the backward half, so every (t, b) slice is a
ready-to-use matmul lhsT [K(65 partitions), M(65)] and each chunk of
timesteps is one contiguous DMA.  Per step and direction: 4 matmuls
(stationary = exp'd energy, moving = state column [65, 1], one PSUM bank
per step) then ONE DVE copy PSUM->SBUF [65, 4].  The 8 chains (4 batch x
2 directions) are independent, so their serial MM->copy->MM latencies
overlap across engines.

tgt path: host precomputes flat gather indices, device gathers via
indirect DMA, masks, reduces, and folds into the loss.
"""

import os
import numpy as np
from contextlib import ExitStack

B, L, NL = 32, 512, 65
H = L // 2
NCORES = 8
BPC = B // NCORES                      # batch per core = 4
CH = 16                                # timesteps per DMA/exp chunk
CBIAS = float(np.float32(np.log(NL) + 0.5))

# gather arrangement: 2048 = 128 partitions x 16 cols;
# partition p = b*32 + r, col g, with t = g*32 + r
GCOLS = L // 32                        # 16

_CACHE = {}

# populated by the last kernel() call when CRF_TRACE=1
last_exec_ns = None
last_profile = None


def _build_program():
    from concourse import bacc, mybir, tile
    import concourse.bass as bass

    f16 = mybir.dt.float16
    f32 = mybir.dt.float32
    i32 = mybir.dt.int32
    Alu = mybir.AluOpType
    Act = mybir.ActivationFunctionType

    nc = bacc.Bacc("TRN2", target_bir_lowering=False, debug=False,
                   num_devices=NCORES)

    energy_h = nc.dram_tensor("energy", [NL, L, BPC, NL], f16,
                              kind="ExternalInput")
    gidx_h = nc.dram_tensor("gidx", [128, GCOLS], i32, kind="ExternalInput")
    gmask_h = nc.dram_tensor("gmask", [128, GCOLS], f32, kind="ExternalInput")
    initu_h = nc.dram_tensor("initu", [NL, BPC], f16, kind="ExternalInput")
    initw_h = nc.dram_tensor("initw", [NL, BPC], f16, kind="ExternalInput")
    bones_h = nc.dram_tensor("bones", [128, BPC], f32, kind="ExternalInput")
    loss_h = nc.dram_tensor("loss", [1, BPC], f32, kind="ExternalOutput")

    energy = energy_h.ap()

    with tile.TileContext(nc) as tc, ExitStack() as ctx:
        cpool = ctx.enter_context(tc.tile_pool(name="consts", bufs=1))
        erawf = ctx.enter_context(tc.tile_pool(name="erawf", bufs=5))
        eexpf = ctx.enter_context(tc.tile_pool(name="eexpf", bufs=3))
        erawb = ctx.enter_context(tc.tile_pool(name="erawb", bufs=5))
        eexpb = ctx.enter_context(tc.tile_pool(name="eexpb", bufs=3))
        uw0 = ctx.enter_context(tc.tile_pool(name="uw0", bufs=2))
        uw1 = ctx.enter_context(tc.tile_pool(name="uw1", bufs=2))
        uw2 = ctx.enter_context(tc.tile_pool(name="uw2", bufs=2))
        ps0 = ctx.enter_context(tc.tile_pool(name="ps0", bufs=2, space="PSUM"))
        ps1 = ctx.enter_context(tc.tile_pool(name="ps1", bufs=2, space="PSUM"))
        ps2 = ctx.enter_context(tc.tile_pool(name="ps2", bufs=2, space="PSUM"))
        psaux = ctx.enter_context(tc.tile_pool(name="psaux", bufs=1, space="PSUM"))
        uwpools = [uw0, uw1, uw2]
        pspools = [ps0, ps1, ps2]

        initu_t = cpool.tile([NL, BPC], f16)
        nc.sync.dma_start(out=initu_t[:], in_=initu_h.ap())
        initw_t = cpool.tile([NL, BPC], f16)
        nc.sync.dma_start(out=initw_t[:], in_=initw_h.ap())
        bones_t = cpool.tile([128, BPC], f32)
        nc.sync.dma_start(out=bones_t[:], in_=bones_h.ap())
        bias_t = cpool.tile([NL, 1], f32)
        nc.vector.memset(bias_t[:], -CBIAS)

        # ---- main recurrence: 2 independent chain groups (fwd u, bwd w) ----
        # Chunk sizes: small priming chunks so the first matmuls start as
        # early as possible, then steady CH-sized chunks.
        sizes = [4, 4, 8] + [CH] * ((H - 16) // CH)
        assert sum(sizes) == H
        starts = list(np.cumsum([0] + sizes[:-1]))

        def pieces_fwd(size):
            # exp piece boundaries (local t, ascending consumption)
            return [(0, size // 2), (size // 2, size - size // 2)] \
                if size >= 8 else [(0, size)]

        def pieces_bwd(size):
            # descending consumption; asymmetric split so bwd exp
            # boundaries do not line up with fwd ones (de-synchronizes
            # the two chains' stall points on the shared DVE queue)
            if size < 8:
                return [(0, size)]
            q = size // 4
            return [(size - q, q), (q, size - 2 * q), (0, q)]

        # Three cast-groups of (3, 3, 2) chains: shorter matmul-group spans
        # on the cast->MMs->cast latency cycle than 2 groups of 4, while
        # keeping DVE cast throughput (3 casts/slot) just under the slot.
        # Each chain (direction, b) lives in one group permanently.
        GROUPS = [[('u', 0), ('u', 1), ('w', 0)],
                  [('u', 2), ('u', 3), ('w', 1)],
                  [('w', 2), ('w', 3)]]
        slotmap = {}
        for g, cols in enumerate(GROUPS):
            for ci, key in enumerate(cols):
                slotmap[key] = (g, ci)
        cur = [None, None, None]

        def state_ap(kind, b):
            g, ci = slotmap[(kind, b)]
            if cur[g] is None:
                base = initu_t if kind == 'u' else initw_t
                return base[:, b:b + 1]
            return cur[g][:, ci:ci + 1]

        for c, (t0, size) in enumerate(zip(starts, sizes)):
            erf = erawf.tile([NL, size, BPC, NL], f16, tag="erf")
            nc.sync.dma_start(out=erf[:], in_=energy[:, t0:t0 + size, :, :])
            fw_pieces = []                       # (lo, hi, tile)
            for pi, (lo, ln) in enumerate(pieces_fwd(size)):
                ee = eexpf.tile([NL, ln, BPC, NL], f16, tag=f"eef{pi}")
                nc.scalar.activation(ee[:], erf[:, lo:lo + ln], Act.Exp,
                                     bias=bias_t[:], scale=1.0)
                fw_pieces.append((lo, lo + ln, ee))

            bt0 = L - t0 - size
            erb = erawb.tile([NL, size, BPC, NL], f16, tag="erb")
            nc.sync.dma_start(out=erb[:], in_=energy[:, bt0:bt0 + size, :, :])
            bw_pieces = []
            for pi, (lo, ln) in enumerate(pieces_bwd(size)):
                ee = eexpb.tile([NL, ln, BPC, NL], f16, tag=f"eeb{pi}")
                nc.scalar.activation(ee[:], erb[:, lo:lo + ln], Act.Exp,
                                     bias=bias_t[:], scale=1.0)
                bw_pieces.append((lo, lo + ln, ee))

            def piece_at(pieces, idx):
                for lo, hi, tl in pieces:
                    if lo <= idx < hi:
                        return tl, idx - lo
                raise AssertionError

            for s in range(size):
                # Colocated cast-groups with a fixed cast rotation on the
                # DVE FIFO: the rotation pins a deterministic phase (no
                # drift into the serialized attractor).
                eef, sf = piece_at(fw_pieces, s)
                eeb, sbl = piece_at(bw_pieces, size - 1 - s)
                new_ps = []
                for g, cols in enumerate(GROUPS):
                    ps = pspools[g].tile([NL, len(cols)], f32)
                    for ci, (kind, b) in enumerate(cols):
                        if kind == 'u':
                            lhsT = eef[:, sf, b, :]
                        else:
                            lhsT = eeb[:, sbl, b, :]
                        nc.tensor.matmul(ps[:, ci:ci + 1], lhsT=lhsT,
                                         rhs=state_ap(kind, b),
                                         start=True, stop=True)
                    new_ps.append(ps)
                newcur = []
                for g, cols in enumerate(GROUPS):
                    uw = uwpools[g].tile([NL, len(cols)], f16)
                    nc.vector.tensor_copy(out=uw[:], in_=new_ps[g][:])
                    newcur.append(uw)
                cur = newcur

            if c == 8:
                # ---- target-energy path: gather 4x512 scalars, mask, reduce
                # The gidx DMA is enqueued here (sync-queue DMAs run in queue
                # order) so the indirect gathers cannot start during the
                # pipeline ramp-up and steal SDMA throughput.
                gidx_t = cpool.tile([128, GCOLS], i32)
                nc.sync.dma_start(out=gidx_t[:], in_=gidx_h.ap())
                gmask_t = cpool.tile([128, GCOLS], f32)
                nc.sync.dma_start(out=gmask_t[:], in_=gmask_h.ap())
                eflat = energy.rearrange("i t b j -> (i t b j)")[:, None]
                gath_t = cpool.tile([128, GCOLS], f16)
                for g in range(GCOLS):
                    nc.gpsimd.indirect_dma_start(
                        out=gath_t[:, g:g + 1],
                        out_offset=None,
                        in_=eflat,
                        in_offset=bass.IndirectOffsetOnAxis(
                            ap=gidx_t[:, g:g + 1], axis=0),
                    )
                gm_t = cpool.tile([128, GCOLS], f32)
                nc.vector.tensor_tensor(out=gm_t[:], in0=gath_t[:],
                                        in1=gmask_t[:], op=Alu.mult)
                gred_t = cpool.tile([128, 1], f32)
                nc.vector.tensor_reduce(out=gred_t[:], in_=gm_t[:],
                                        axis=mybir.AxisListType.X, op=Alu.add)
                tgt_ps = psaux.tile([1, BPC], f32)
                nc.tensor.matmul(tgt_ps[:], lhsT=gred_t[:], rhs=bones_t[:],
                                 start=True, stop=True)
                tgt_sb = cpool.tile([1, BPC], f32)
                nc.vector.tensor_copy(out=tgt_sb[:], in_=tgt_ps[:])

        # ---- epilogue: loss = ln(u^T w) + L*c - tgt ----
        s_ps = psaux.tile([1, BPC], f32)
        for b in range(BPC):
            nc.tensor.matmul(s_ps[:, b:b + 1], lhsT=state_ap('u', b),
                             rhs=state_ap('w', b), start=True, stop=True)
        lsb = cpool.tile([1, BPC], f32)
        nc.scalar.activation(lsb[:], s_ps[:], Act.Ln, bias=0.0, scale=1.0)
        nc.vector.tensor_tensor(out=lsb[:], in0=lsb[:], in1=tgt_sb[:],
                                op=Alu.subtract)
        nc.vector.tensor_scalar_add(lsb[:], lsb[:], float(L) * CBIAS)
        nc.sync.dma_start(out=loss_h.ap(), in_=lsb[:])

    nc.compile()
    return nc


def _get_program():
    if "nc" not in _CACHE:
        _CACHE["nc"] = _build_program()
    return _CACHE["nc"]


def _prep_inputs(energy, target, mask):
    """Host-side sharding + layout. Returns in_maps (one dict per core)."""
    energy = np.asarray(energy, dtype=np.float32)
    target = np.asarray(target).astype(np.int64)
    mask = np.asarray(mask, dtype=np.float32)

    all_ones = bool(np.all(mask == 1.0))
    if all_ones:
        energy_eff = energy
        gmask_full = np.ones((B, L), np.float32)
    else:
        # binary-mask general path: masked steps (t>0) become identity
        # transitions after exp(x - c); masked t=0 stays the zero matrix.
        energy_eff = energy * mask[:, :, None, None]
        sub = np.full((NL, NL), -1e4, np.float32)
        np.fill_diagonal(sub, CBIAS)
        zb, zt = np.nonzero(mask == 0.0)
        for bb, tt in zip(zb, zt):
            if tt > 0:
                energy_eff[bb, tt] = sub
        gmask_full = (mask != 0.0).astype(np.float32)

    initu = np.zeros((NL, BPC), np.float16)
    initu[NL - 1, :] = 1.0
    initw = np.ones((NL, BPC), np.float16)
    bones = np.zeros((128, BPC), np.float32)
    for b in range(BPC):
        bones[b * 32:(b + 1) * 32, b] = 1.0

    in_maps = []
    for k in range(NCORES):
        sl = slice(k * BPC, (k + 1) * BPC)
        eb = energy_eff[sl]                                   # [4, L, 65, 65]
        et = np.empty((NL, L, BPC, NL), np.float16)
        et[:, :H] = eb[:, :H].transpose(2, 1, 0, 3)           # fwd: [i, t, b, j]
        et[:, H:] = eb[:, H:].transpose(3, 1, 0, 2)           # bwd: [j, t, b, i]

        tg = target[sl]                                       # [4, L]
        mk = gmask_full[sl]
        prev = np.concatenate(
            [np.full((BPC, 1), NL - 1, np.int64), tg[:, :-1]], axis=1)
        tt = np.arange(L, dtype=np.int64)[None, :]
        bb = np.arange(BPC, dtype=np.int64)[:, None]
        # fwd layout [i=prev, t, b, j=cur]; bwd layout [j=cur, t, b, i=prev]
        flat_f = ((prev * L + tt) * BPC + bb) * NL + tg
        flat_b = ((tg * L + tt) * BPC + bb) * NL + prev
        flat = np.where(tt < H, flat_f, flat_b)               # [4, L]

        gidx = np.zeros((128, GCOLS), np.int32)
        gmask = np.zeros((128, GCOLS), np.float32)
        for b in range(BPC):
            # partition p = b*32 + r holds t = g*32 + r at column g
            gidx[b * 32:(b + 1) * 32, :] = flat[b].reshape(GCOLS, 32).T
            gmask[b * 32:(b + 1) * 32, :] = mk[b].reshape(GCOLS, 32).T

        in_maps.append({
            "energy": et,
            "gidx": gidx,
            "gmask": gmask,
            "initu": initu,
            "initw": initw,
            "bones": bones,
        })
    return in_maps


def _install_ntff_hook_shim():
    """The agent image's antenv lacks axon_hooks; synthesize it so
    run_bass_kernel_spmd(trace=True) can find the NTFF profile hook."""
    import sys
    import types
    try:
        import antenv.axon_hooks  # noqa: F401
        return
    except ImportError:
        pass
    import antenv
    mod = types.ModuleType("antenv.axon_hooks")
    _h = [None]
    mod.set_axon_ntff_profile_hook = lambda h: _h.__setitem__(0, h)
    mod.get_axon_ntff_profile_hook = lambda: _h[0]
    sys.modules["antenv.axon_hooks"] = mod
    antenv.axon_hooks = mod
    try:
        from trn_agent_boot.trn_boot import _ntff_profile_via_ctypes
        hook = _ntff_profile_via_ctypes("/opt/axon/libaxon_pjrt.so")
        if hook is not None:
            mod.set_axon_ntff_profile_hook(hook)
    except Exception:
        pass


def kernel(energy, target, mask):
    global last_exec_ns, last_profile
    from concourse.bass_utils import run_bass_kernel_spmd

    nc = _get_program()
    in_maps = _prep_inputs(energy, target, mask)
    trace = bool(int(os.environ.get("CRF_TRACE", "0")))
    if trace:
        _install_ntff_hook_shim()
    res = run_bass_kernel_spmd(nc, in_maps, list(range(NCORES)), trace=trace)
    last_exec_ns = res.exec_time_ns
    last_profile = res.profile_json
    out = np.concatenate(
        [res.results[k]["loss"].reshape(BPC) for k in range(NCORES)])
    return out.astype(np.float32)
